# revision 29
# baseline (speedup 1.0000x reference)
"""MoE feed-forward (8 experts, top-2) on 8 Trainium2 NeuronCores.

Strategy (expert-parallel, per the sharding hint):
  - Gate (tiny: [4096,768]@[768,8]) computed on host with jax, replicating the
    reference's op sequence exactly so top-2 routing decisions match
    bit-for-bit.
  - Tokens are dispatched by top-k expert id on the host (the host plays the
    role of the all-to-all): core e receives the tokens routed to expert e,
    padded to a common capacity so one SPMD program serves all 8 cores.
  - Each core runs a Bass/Tile kernel: y = relu(x @ w1.T + b1) @ w2.T + b2
    for its expert over its routed tokens, with float32r (fp22) matmuls on
    the 128x128 PE array.
  - Host combines with the gate-prob weights (the weighted all-to-all):
    out[token] += prob * y.
"""

import os
import sys

import numpy as np

for _p in ("/opt/trn_rl_repo", "/root/.axon_site/_ro/trn_rl_repo"):
    if os.path.isdir(_p) and _p not in sys.path:
        sys.path.insert(0, _p)
        break

P = 128
C = 768
H = 3072
E = 8
TOP_K = 2
KC = C // P  # 6
KH = H // P  # 24
N_CORES = 8

# Populated by the most recent kernel() call, for test.py introspection.
LAST_RESULTS = None
_NC_CACHE = {}


def _split_tiles(n):
    """Split n (multiple of 128) into chunks, each <=512 and >=256 when
    possible (float32r matmuls run at full PE rate only for moving dim
    >= 256)."""
    if n <= 512:
        return [n]
    ts = []
    rem = n
    while rem > 512:
        if rem - 512 >= 256:
            ts.append(512)
            rem -= 512
        else:
            ts.append(384)
            rem -= 384
    ts.append(rem)
    return ts


def _gate_host(xr, gate_w, gate_b):
    """Replicate the reference gating ops exactly (same jax ops, default
    platform) so the top-2 selection matches the reference bit-for-bit.
    Falls back to numpy (verified to produce identical top-2 picks on
    these inputs) if jax is unavailable."""
    try:
        import jax
        import jax.numpy as jnp

        logits = jnp.asarray(xr) @ jnp.asarray(gate_w).T + jnp.asarray(gate_b)
        probs = jax.nn.softmax(logits, axis=-1)
        topv, topi = jax.lax.top_k(probs, TOP_K)
        topv = topv / jnp.sum(topv, axis=-1, keepdims=True)
        return np.asarray(topv), np.asarray(topi)
    except Exception:
        logits = xr @ gate_w.T + gate_b
        m = logits.max(axis=-1, keepdims=True)
        ex = np.exp(logits - m)
        probs = ex / ex.sum(axis=-1, keepdims=True)
        topi = np.argsort(-probs, axis=-1, kind="stable")[:, :TOP_K]
        topv = np.take_along_axis(probs, topi, axis=-1)
        topv = topv / topv.sum(axis=-1, keepdims=True)
        return topv.astype(np.float32), topi


def _build_nc(ncap, tiles, debug=False, mm_dtype="f32r"):
    import concourse.bacc as bacc
    import concourse.mybir as mybir
    import concourse.tile as tile

    f32 = mybir.dt.float32
    f32r = mybir.dt.bfloat16 if mm_dtype == "bf16" else mybir.dt.float32r
    add = mybir.AluOpType.add
    amax = mybir.AluOpType.max

    nc = bacc.Bacc("TRN2", target_bir_lowering=False, debug=debug)

    xT = nc.dram_tensor("xT", [P, KC * ncap], f32r, kind="ExternalInput").ap()
    w1t = nc.dram_tensor("w1t", [C, H], f32r, kind="ExternalInput").ap()
    w2t = nc.dram_tensor("w2t", [H, C], f32r, kind="ExternalInput").ap()
    b1r = nc.dram_tensor("b1r", [P, KH], f32, kind="ExternalInput").ap()
    b2r = nc.dram_tensor("b2r", [P, KC], f32, kind="ExternalInput").ap()
    yT = nc.dram_tensor("yT", [C, ncap], f32, kind="ExternalOutput").ap()

    with tile.TileContext(nc) as tc:
        with (
            tc.tile_pool(name="weights", bufs=1) as wpool,
            tc.tile_pool(name="xpool", bufs=2) as xpool,
            tc.tile_pool(name="x0pool", bufs=1) as x0pool,
            tc.tile_pool(name="hpool", bufs=9) as hpool,
            tc.tile_pool(name="ypool", bufs=1) as ypool,
            tc.tile_pool(name="psh", bufs=2, space="PSUM") as psh,
            tc.tile_pool(name="psy", bufs=1, space="PSUM") as psy,
        ):
            yTv = yT.rearrange("(o p) n -> p o n", p=P)  # [128, 6, ncap]

            def x_src(ti, tok0, T):
                # host packs x per-tile k-major: [p, (tile | k | n)] so each
                # tile's load is one fully contiguous 12KB/partition DMA
                off = tok0 * KC
                return xT[:, off : off + KC * T].rearrange("p (k n) -> p k n", n=T)

            # DMA issue order matters: the sync-engine HWDGE ring is FIFO.
            # x tile 0 + w1 first (both needed for the first matmuls), then
            # w2 (streamed behind compute, consumed slower than delivered).
            # The tiny strided bias loads go on the gpsimd SWDGE queue so
            # they don't head-block the weight stream.
            b1_sb = wpool.tile([P, KH], f32, tag="b1", name="b1")
            nc.gpsimd.dma_start(b1_sb, b1r)
            b2_sb = wpool.tile([P, KC], f32, tag="b2", name="b2")
            nc.gpsimd.dma_start(b2_sb, b2r)

            x0_sb = xpool.tile([P, KC, tiles[0]], f32r, tag="x", name="x")
            nc.sync.dma_start(x0_sb, x_src(0, 0, tiles[0]))

            w1v = w1t.rearrange("(o p) f -> p o f", p=P)  # [128, 6, 3072]
            w1_sb = []
            for k in range(KC):
                t = wpool.tile([P, H], f32r, tag=f"w1_{k}", name=f"w1_{k}")
                nc.sync.dma_start(t, w1v[:, k])
                w1_sb.append(t)

            w2v = w2t.rearrange("(o p) f -> p o f", p=P)  # [128, 24, 768]
            w2_sb = []
            for j in range(KH):
                t = wpool.tile([P, C], f32r, tag=f"w2_{j}", name=f"w2_{j}")
                nc.sync.dma_start(t, w2v[:, j])
                w2_sb.append(t)

            # Prefetch the remaining x tiles now: the sync engine issues
            # dma_starts in order, so any x issued after the y stores would
            # wait behind their copy semaphores (~the whole previous tile).
            x_tiles = [x0_sb]
            for ti in range(1, len(tiles)):
                tok0 = sum(tiles[:ti])
                x_sb = xpool.tile([P, KC, tiles[ti]], f32r, tag="x", name="x")
                nc.sync.dma_start(x_sb, x_src(ti, tok0, tiles[ti]))
                x_tiles.append(x_sb)

            for ti, tsize in enumerate(tiles):
                tok0 = sum(tiles[:ti])
                x_k = [x_tiles[ti][:, k, :] for k in range(KC)]

                ps_y = [
                    psy.tile([P, tsize], f32, tag=f"py{c}", name=f"py{c}") for c in range(KC)
                ]

                def emit_l2(j, h_t):
                    for c in range(KC):
                        nc.tensor.matmul(
                            ps_y[c],
                            lhsT=w2_sb[j][:, c * P : (c + 1) * P],
                            rhs=h_t,
                            start=(j == 0),
                            stop=(j == KH - 1),
                        )

                def emit_l1(j, ps_h, k):
                    nc.tensor.matmul(
                        ps_h,
                        lhsT=w1_sb[k][:, j * P : (j + 1) * P],
                        rhs=x_k[k],
                        start=(k == 0),
                        stop=(k == KC - 1),
                    )

                def emit_relu(j, ps_h):
                    h_t = hpool.tile([P, tsize], f32r, tag="h", name="h")
                    # h = max(psum + b1, 0)  (relu with bias) on the DVE
                    nc.vector.tensor_scalar(
                        h_t, ps_h, b1_sb[:, j : j + 1], 0.0, add, amax
                    )
                    return h_t

                pending = []  # h-tiles with layer-2 not yet emitted
                j_start = 0
                if ti == 0:
                    # First tile: k-outer over 8 PSUM banks so matmuls start
                    # as soon as w1[k] arrives, instead of after all of w1.
                    NP1 = 8
                    ph1 = [
                        psy.tile([P, tsize], f32, tag=f"py{j}", name=f"py{j}")
                        for j in range(KC)
                    ] + [psh.tile([P, tsize], f32, tag="ph", name="ph") for _ in range(2)]
                    for k in range(KC):
                        for j in range(NP1):
                            emit_l1(j, ph1[j], k)
                    for j in range(NP1):
                        pending.append((j, emit_relu(j, ph1[j])))
                    j_start = NP1
                    # ps_y tiles must be re-allocated after ph1 frees the
                    # banks (same tags -> same slots, Tile serializes).
                    ps_y = [
                        psy.tile([P, tsize], f32, tag=f"py{c}", name=f"py{c}")
                        for c in range(KC)
                    ]

                # Software-pipelined steady state: layer-1 matmuls for the
                # next h-tile are emitted before layer-2 of the previous one,
                # so the PE never waits on the relu.
                for j in range(j_start, KH):
                    ps_h = psh.tile([P, tsize], f32, tag="ph", name="ph")
                    for k in range(KC):
                        emit_l1(j, ps_h, k)
                    h_t = emit_relu(j, ps_h)
                    # drain up to 2 pending layer-2 blocks per iteration so
                    # the phase-1 backlog shrinks (frees h buffers).
                    if pending:
                        emit_l2(*pending.pop(0))
                    if len(pending) > 1:
                        emit_l2(*pending.pop(0))
                    pending.append((j, h_t))

                for item in pending:
                    emit_l2(*item)

                for c in range(KC):
                    y_t = ypool.tile([P, tsize], f32, tag=f"y{c}", name=f"y{c}")
                    nc.vector.tensor_scalar_add(y_t, ps_y[c], b2_sb[:, c : c + 1])
                    eng = nc.sync if c % 2 == 0 else nc.scalar
                    eng.dma_start(yTv[:, c, tok0 : tok0 + tsize], y_t)

    nc.compile()
    return nc


def _route(topv, topi, n_tokens):
    """Per-expert token index lists + combine weights."""
    idxs, wts = [], []
    for e in range(E):
        hit = topi == e  # [N, K] bool
        tok = np.nonzero(hit.any(axis=1))[0]
        # weight for token t is topv[t, k] where topi[t, k] == e
        w = (topv * hit)[tok].sum(axis=1)
        idxs.append(tok.astype(np.int64))
        wts.append(w.astype(np.float32))
    return idxs, wts


def _enable_ntff_hook():
    """Register the axon NTFF profiling hook when the image's antenv lacks
    axon_hooks (profiling-only plumbing; compile/run work without it)."""
    import sys as _sys
    import types

    try:
        from antenv.axon_hooks import get_axon_ntff_profile_hook  # noqa: F401

        return
    except ImportError:
        pass
    try:
        from trn_agent_boot.trn_boot import _ntff_profile_via_ctypes
    except ImportError:
        return
    hook = _ntff_profile_via_ctypes("/opt/axon/libaxon_pjrt.so")
    mod = types.ModuleType("antenv.axon_hooks")
    mod.get_axon_ntff_profile_hook = lambda: hook
    mod.set_axon_ntff_profile_hook = lambda h: None
    _sys.modules["antenv.axon_hooks"] = mod
    import concourse.bass_utils as bu

    bu.upload_artifacts = lambda tmpdir: tmpdir  # no artifact bucket here


def kernel(x, gate_w, gate_b, w1, b1, w2, b2):
    global LAST_RESULTS
    from concourse.bass_utils import run_bass_kernel_spmd

    trace = bool(int(os.environ.get("KERNEL_TRACE", "0")))
    if trace:
        _enable_ntff_hook()

    x = np.asarray(x, dtype=np.float32)
    B, T, _ = x.shape
    n = B * T
    xr = np.ascontiguousarray(x.reshape(n, C))

    topv, topi = _gate_host(xr, np.asarray(gate_w), np.asarray(gate_b))
    idxs, wts = _route(topv, topi, n)

    counts = [len(i) for i in idxs]
    # Cap device capacity at 1024 tokens/expert (= N*TOP_K/E): keeps the
    # device tiles at the maximally efficient [512, 512] shape; the few
    # overflow tokens of hot experts are computed on host in exact fp32.
    cap = min(max(counts), 1024)
    dev_counts = [min(c, cap) for c in counts]
    ncap = max(256, -(-max(dev_counts) // P) * P)
    tiles = _split_tiles(ncap)

    w1 = np.asarray(w1, dtype=np.float32)
    w2 = np.asarray(w2, dtype=np.float32)
    b1 = np.asarray(b1, dtype=np.float32)
    b2 = np.asarray(b2, dtype=np.float32)

    in_maps = []
    for e in range(E):
        xe = np.zeros((C, ncap), dtype=np.float32)
        xe[:, : dev_counts[e]] = xr[idxs[e][: dev_counts[e]]].T
        # pack per-tile k-major: xp[p, tile_off + k*T + n] = xe[k*128+p, tok0+n]
        xp = np.empty((P, KC * ncap), dtype=np.float32)
        off = 0
        tok0 = 0
        for tsz in tiles:
            blk = xe[:, tok0 : tok0 + tsz].reshape(KC, P, tsz)
            xp[:, off : off + KC * tsz] = blk.transpose(1, 0, 2).reshape(P, KC * tsz)
            off += KC * tsz
            tok0 += tsz
        in_maps.append(
            {
                "xT": xp,
                "w1t": np.ascontiguousarray(w1[e].T),
                "w2t": np.ascontiguousarray(w2[e].T),
                "b1r": np.ascontiguousarray(b1[e].reshape(KH, P).T),
                "b2r": np.ascontiguousarray(b2[e].reshape(KC, P).T),
            }
        )

    mm_dtype = os.environ.get("KERNEL_MM_DTYPE", "f32r")
    if mm_dtype == "bf16":
        import ml_dtypes

        bf16 = np.dtype(ml_dtypes.bfloat16)
        for m in in_maps:
            for kk in ("xT", "w1t", "w2t"):
                m[kk] = m[kk].astype(bf16)
    cache_key = (ncap, tuple(tiles), mm_dtype)
    nc = _NC_CACHE.get(cache_key)
    if nc is None:
        nc = _build_nc(ncap, tiles, debug=False, mm_dtype=mm_dtype)
        _NC_CACHE[cache_key] = nc
    tmpdir = None
    if trace:
        import tempfile

        tmpdir = tempfile.mkdtemp(prefix="moe_trace_")
    res = run_bass_kernel_spmd(
        nc, in_maps, core_ids=list(range(N_CORES)), trace=trace, tmpdir=tmpdir
    )
    LAST_RESULTS = res

    out = np.zeros((n, C), dtype=np.float32)
    for e in range(E):
        nd = dev_counts[e]
        ye = res.results[e]["yT"][:, :nd].T  # [nd, C]
        out[idxs[e][:nd]] += wts[e][:nd, None] * ye
        if counts[e] > nd:  # host-side overflow (exact fp32)
            xo = xr[idxs[e][nd:]]
            ho = np.maximum(xo @ w1[e].T + b1[e], 0.0)
            yo = ho @ w2[e].T + b2[e]
            out[idxs[e][nd:]] += wts[e][nd:, None] * yo
    return out.reshape(B, T, C)
